# revision 3
# baseline (speedup 1.0000x reference)
"""ConditionEmbedder kernel for 8 Trainium2 NeuronCores.

Math (train=0, unconditioned=0 path):
    drop = isnan(labels);  safe = where(drop, 0, labels)
    s    = softmax(safe[:,d,None]*w1[d] + b1[d], axis=-1)        # per (b, d)
    mlp  = s @ w2[d].T
    out  = sum_d where(drop, emb_w[d], mlp)                      # [B, H]

Approach: the per-dim contribution f_d(x) = w2[d] @ softmax(x*w1[d]+b1[d])
is a smooth vector-valued function of ONE scalar.  Host fits each f_d with a
degree-8 Chebyshev series in the warped variable v = tanh(alpha_d*x/S) /
tanh(alpha_d), so the whole batch reduces to K=73 matmuls:

    out[k, b] = sum_{d} [ (C1[d]/2) * wm_{b,d}                  (rows 0..7)
              + sum_{p=2..8} C[d,p,k] * T_p(v_{b,d})            (rows 8..63)
              + drop_{b,d} * (emb_w[d,k] - fhat_d(-1,k))        (rows 64..71)
              ] + sum_d C[d,0,k]                                (ones row 72)

where wm = 2v.  NaN labels propagate through tanh and are steered to
wm = -2 (v = -1) by a vector max; every T_p(-1) = (-1)^p is exact in fp16,
and the drop rows supply emb_w - fhat(-1).

Device pipeline (pure data parallel over batch, 8 cores), fp16 throughout:
  * scalar: tanh (per-partition scale alpha_d/S), x2/tanh(alpha) identity,
    squares for even-degree doubling; half the PSUM->SBUF downcasts.
  * vector: max NaN-kill, Chebyshev chain T2..T8; other half of downcasts.
  * gpsimd: drop mask (x != x), scatter dispatches.
  * DMA scatters build the [73, 16384] fp16 moving tile (row 8(p-1)+d);
    two column groups pipeline the chain against the matmul phase.
  * tensor: paced junk matmuls keep the PE p-state high, then 32 K=73
    fp16 matmuls [73,128] x [73,512].
  * stores: 8 DMAs of 1 MB, [128, 4x512-col blocks] fp16.
Host: Chebyshev fit, label relayout to row 16d+c, output transpose/upcast.
"""

import sys

import numpy as np

_B, _D, _H = 131072, 8, 128
_NCORES = 8
_BC = _B // _NCORES          # batch rows per core
_P = 8                       # Chebyshev degree
_KROWS = 8 * _P + 8 + 1      # 64 cheb + 8 drop + 1 ones = 73
_NC_CHUNK = 16               # c-chunks per core (rows 16d+c)
_G = 2                       # column pipeline groups
_GW = 1024 // _G             # label-tile columns per group


def _np_reference(labels, emb_w, w1, b1, w2, train, unconditioned):
    """Slow exact fallback for the train/unconditioned branches (uses jax to
    reproduce the reference PRNG streams)."""
    import jax
    import jax.numpy as jnp

    DROPOUT_PROB = 0.1
    labels = jnp.asarray(labels)
    if unconditioned:
        drop = jnp.ones(labels.shape, dtype=bool)
    else:
        drop = jnp.isnan(labels)
        if train:
            rkey = jax.random.fold_in(jax.random.key(0), 1)
            drop = drop | (jax.random.uniform(rkey, labels.shape) < DROPOUT_PROB)
    safe = jnp.where(drop, 0.0, labels)
    h1 = safe[:, :, None] * w1[None, :, :] + b1[None, :, :]
    s = jax.nn.softmax(h1, axis=-1)
    mlp = jnp.einsum('bdh,dkh->bdk', s, w2)
    emb = jnp.where(drop[:, :, None], emb_w[None, :, :], mlp)
    if train:
        nkey = jax.random.fold_in(jax.random.key(0), 2)
        emb = emb + jax.random.normal(nkey, emb.shape, dtype=emb.dtype)
    return np.asarray(emb.sum(axis=1))


def _fit_cheb(emb_w, w1, b1, w2, S):
    """Fit f_d(x) = w2[d] @ softmax(x*w1[d]+b1[d]) with a degree-P Chebyshev
    series in v = tanh(alpha_d*x/S)/tanh(alpha_d), alpha_d by grid search.

    Returns (chebT [128,128] f16 stationary (rows 0.._KROWS-1 used),
    alpha [8] f64)."""
    import numpy.polynomial.chebyshev as CH

    G = 4097
    u = np.linspace(-1.0, 1.0, G)
    alphas = np.arange(1.0, 2.45, 0.05)
    C = np.zeros((_D, _P + 1, _H))
    alpha = np.zeros(_D)
    for d in range(_D):
        lg = (S * u)[:, None] * w1[d][None, :].astype(np.float64) \
            + b1[d][None, :].astype(np.float64)
        m = lg.max(-1, keepdims=True)
        e = np.exp(lg - m)
        s = e / e.sum(-1, keepdims=True)
        F = s @ w2[d].T.astype(np.float64)          # [G, H]
        best = None
        for a in alphas:
            v = np.tanh(a * u) / np.tanh(a)
            Cd = CH.chebfit(v, F, _P)
            r = np.abs(CH.chebval(v, Cd) - F.T).max()
            if best is None or r < best[0]:
                best = (r, a, Cd)
        alpha[d] = best[1]
        C[d] = best[2]

    f16 = np.float16
    chebT = np.zeros((128, 128), f16)
    for d in range(_D):
        chebT[d, :] = (C[d, 1, :] * 0.5).astype(f16)            # wm row
        for p in range(2, _P + 1):
            chebT[8 * (p - 1) + d, :] = C[d, p, :].astype(f16)  # T_p rows
    # drop rows: emb_w - (device value at v=-1, computed from the f16
    # coefficients exactly: wm=-2 on row d, T_p(-1)=(-1)^p)
    for d in range(_D):
        fhat_m1 = (-2.0) * chebT[d, :].astype(np.float64)
        for p in range(2, _P + 1):
            fhat_m1 += chebT[8 * (p - 1) + d, :].astype(np.float64) * (-1.0) ** p
        fhat_m1 += C[d, 0, :]
        chebT[64 + d, :] = (emb_w[d].astype(np.float64) - fhat_m1).astype(f16)
    chebT[72, :] = C[:, 0, :].sum(0).astype(f16)                # ones/bias row
    return chebT, alpha


class _Builder:
    """Builds the per-core Bass program (identical on all cores; data differs)."""

    def __init__(self):
        sys.path.insert(0, '/opt/trn_rl_repo')
        import concourse.mybir as mybir
        from concourse import bass, tile
        from concourse.vector_clock import ScopedClock

        self.mybir = mybir
        self.bass = bass
        self.tile = tile
        self.ScopedClock = ScopedClock

    def make_tile_context(self, nc):
        mybir = self.mybir
        tile = self.tile
        ScopedClock = self.ScopedClock

        class PatchedTileContext(tile.TileContext):
            # walrus in this container rejects >1 sync-wait on the tail Drain
            # (setupSyncWait CTRL limit); spread the end-of-kernel waits
            # across single-wait SP nops instead.
            def _drain_and_barrier(self, tick_clock, wait_clock):
                nc_ = self.nc
                probe = nc_.sync.nop(nofuse=True)
                wait_clock.add_sem_waits(
                    probe.ins, ScopedClock({None: tick_clock.global_clock})
                )
                si = probe.ins.sync_info
                waits = list(si.on_wait) if si and si.on_wait else []
                if len(waits) > 1:
                    si.on_wait.clear()
                    si.on_wait.append(waits[0])
                    for w in waits[1:]:
                        n2 = nc_.sync.nop(nofuse=True)
                        s2 = n2.ins.sync_info
                        if s2 is None:
                            n2.ins.sync_info = mybir.SyncInfo(on_wait=[w], on_update=[])
                        else:
                            s2.on_wait.append(w)
                nc_.sync.drain()
                nc_.all_engine_barrier()
                assert self.sems is not None
                popped = nc_._tile_sem_poison_stack.pop()
                assert popped is self._sem_poison
                nc_.clear_and_free_semaphores(list(self.sems.allocated().values()))
                nc_.all_engine_barrier()

        return PatchedTileContext(nc)

    def build(self):
        mybir = self.mybir
        bass = self.bass
        dt = mybir.dt
        ALU = mybir.AluOpType
        F32, F16 = dt.float32, dt.float16
        Act = mybir.ActivationFunctionType

        nc = bass.Bass(trn_type="TRN2", enable_partition_id=False)

        # ---- DRAM parameters ----
        # per-core labels, transposed-dense fp16: row (16d + c) holds
        # labels[c*1024:(c+1)*1024, d]  (NaNs preserved)
        p_lab = nc.declare_dram_parameter("lab_td", [128, 1024], F16, isOutput=False)
        p_cheb = nc.declare_dram_parameter("chebT", [128, 128], F16, isOutput=False)
        # warp consts per partition: col0 = alpha_d/S, col1 = 2/tanh(alpha_d)
        p_warp = nc.declare_dram_parameter("warp", [128, 2], F32, isOutput=False)
        p_ones = nc.declare_dram_parameter("onesr", [1, _BC], F16, isOutput=False)
        p_out = nc.declare_dram_parameter("outT", [128, _NC_CHUNK, 1024], F16,
                                          isOutput=True)

        from contextlib import ExitStack

        with self.make_tile_context(nc) as tc, ExitStack() as ctx:
            consts = ctx.enter_context(tc.tile_pool(name="consts", bufs=1))
            prep = ctx.enter_context(tc.tile_pool(name="prep", bufs=1))
            pop = ctx.enter_context(tc.tile_pool(name="pout", bufs=6, space="PSUM"))
            junkp = ctx.enter_context(tc.tile_pool(name="junk", bufs=1, space="PSUM"))
            obp = ctx.enter_context(tc.tile_pool(name="ob", bufs=3))

            # ---- inputs: labels first (critical path) on the sync HWDGE
            # queue; constants on the scalar queue; ones row on gpsimd ----
            t_lab = prep.tile([128, 1024], F16)
            nc.sync.dma_start(t_lab[:], p_lab[:])
            t_warp = consts.tile([128, 2], F32)
            nc.scalar.dma_start(t_warp[:], p_warp[:])
            t_cheb = consts.tile([128, 128], F16)
            nc.scalar.dma_start(t_cheb[:], p_cheb[:])

            # moving tile: row 8(p-1)+d = T_p(v_d), rows 64..71 drop, 72 ones
            t_mov = consts.tile([128, _NC_CHUNK, 1024], F16)
            nc.gpsimd.dma_start(t_mov[72:73, :, :], p_ones[:])

            # ---- junk matmuls keep the PE p-state high until the real burst
            t_junk = junkp.tile([128, 512], F32)

            def warm(src, n=1):
                for _ in range(n):
                    nc.tensor.matmul(t_junk[:], t_cheb[0:6, :], src[0:6, 0:512],
                                     start=True, stop=True,
                                     skip_group_check=True)

            # per-group chain tiles (full [128, 1024]; ops address col slices)
            def mk(name):
                return prep.tile([128, 1024], F16, name=name)

            t_tan, t_wmr, t_wm, t_drop = mk("tan"), mk("wmr"), mk("wm"), mk("drop")
            t_sqw, t_sq2, t_sq3, t_sq4 = mk("sqw"), mk("sq2"), mk("sq3"), mk("sq4")
            t_a3, t_a5, t_a7 = mk("a3"), mk("a5"), mk("a7")
            t_T2, t_T3, t_T4, t_T5 = mk("T2"), mk("T3"), mk("T4"), mk("T5")
            t_T6, t_T7, t_T8 = mk("T6"), mk("T7"), mk("T8")

            # scatter queue rotation: keep the sync queue light early (it has
            # the label load), gpsimd dispatches are cheap SWDGE
            scat_engines = [nc.gpsimd, nc.sync, nc.scalar]

            def chain_group(g, junk=True):
                gs = slice(_GW * g, _GW * (g + 1))
                scat_n = [0]

                def scat(p, src):
                    eng = scat_engines[scat_n[0] % len(scat_engines)]
                    scat_n[0] += 1
                    # dst rows 8(p-1)..+8, cols (c, GW*g..): src row 16d+c
                    eng.dma_start(t_mov[8 * (p - 1):8 * p, :, gs], src[:, gs])

                # scalar: tanh with per-partition scale; NaNs propagate
                nc.scalar.activation(t_tan[:, gs], t_lab[:, gs], Act.Tanh,
                                     scale=t_warp[:, 0:1])
                # vector: drop mask (NaN != NaN -> 1.0)
                nc.vector.tensor_tensor(t_drop[:, gs], t_tan[:, gs], t_tan[:, gs],
                                        ALU.not_equal)
                # scalar: wm_raw = 2*tanh/tanh(alpha)
                nc.scalar.activation(t_wmr[:, gs], t_tan[:, gs], Act.Identity,
                                     scale=t_warp[:, 1:2])
                # vector: NaN -> -2 (max returns the non-NaN operand)
                nc.vector.tensor_scalar_max(t_wm[:, gs], t_wmr[:, gs], -2.0)
                scat(1, t_wm)
                if junk:
                    warm(t_wm)
                nc.scalar.activation(t_sqw[:, gs], t_wm[:, gs], Act.Square)
                nc.vector.tensor_scalar(t_T2[:, gs], t_sqw[:, gs], 0.5, -1.0,
                                        ALU.mult, ALU.add)
                scat(2, t_T2)
                if junk:
                    warm(t_T2)
                nc.vector.tensor_scalar_add(t_a3[:, gs], t_T2[:, gs], -0.5)
                nc.vector.tensor_tensor(t_T3[:, gs], t_wm[:, gs], t_a3[:, gs],
                                        ALU.mult)
                scat(3, t_T3)
                if junk:
                    warm(t_T3)
                nc.scalar.activation(t_sq2[:, gs], t_T2[:, gs], Act.Square)
                nc.vector.tensor_scalar(t_T4[:, gs], t_sq2[:, gs], 2.0, -1.0,
                                        ALU.mult, ALU.add)
                scat(4, t_T4)
                if junk:
                    warm(t_T4)
                nc.vector.tensor_tensor(t_a5[:, gs], t_wm[:, gs], t_T4[:, gs],
                                        ALU.mult)
                nc.vector.tensor_tensor(t_T5[:, gs], t_a5[:, gs], t_T3[:, gs],
                                        ALU.subtract)
                scat(5, t_T5)
                if junk:
                    warm(t_T5)
                nc.scalar.activation(t_sq3[:, gs], t_T3[:, gs], Act.Square)
                nc.vector.tensor_scalar(t_T6[:, gs], t_sq3[:, gs], 2.0, -1.0,
                                        ALU.mult, ALU.add)
                scat(6, t_T6)
                if junk:
                    warm(t_T6)
                nc.vector.tensor_tensor(t_a7[:, gs], t_wm[:, gs], t_T6[:, gs],
                                        ALU.mult)
                nc.vector.tensor_tensor(t_T7[:, gs], t_a7[:, gs], t_T5[:, gs],
                                        ALU.subtract)
                scat(7, t_T7)
                if junk:
                    warm(t_T7)
                nc.scalar.activation(t_sq4[:, gs], t_T4[:, gs], Act.Square)
                nc.vector.tensor_scalar(t_T8[:, gs], t_sq4[:, gs], 2.0, -1.0,
                                        ALU.mult, ALU.add)
                scat(8, t_T8)
                if junk:
                    warm(t_T8)
                # drop rows
                eng = scat_engines[scat_n[0] % len(scat_engines)]
                eng.dma_start(t_mov[8 * _P:8 * _P + 8, :, gs], t_drop[:, gs])

            def mm_group(g):
                gs = slice(_GW * g, _GW * (g + 1))
                ob = None
                for c in range(_NC_CHUNK):
                    po = pop.tile([128, 512], F32, tag="po", name=f"po_{g}_{c}")
                    nc.tensor.matmul(po[:], t_cheb[0:_KROWS, :],
                                     t_mov[0:_KROWS, c, gs],
                                     start=True, stop=True)
                    if c % 4 == 0:
                        ob = obp.tile([128, 4, 512], F16, tag="ob",
                                      name=f"ob_{g}_{c // 4}")
                    # PSUM -> SBUF downcast; alternate engines
                    if c % 2 == 0:
                        nc.scalar.copy(ob[:, c % 4, :], po[:])
                    else:
                        nc.vector.tensor_copy(ob[:, c % 4, :], po[:])
                    if c % 4 == 3:
                        nc.sync.dma_start(
                            p_out[:, c - 3:c + 1, gs], ob[:])

            chain_group(0)
            chain_group(1)
            mm_group(0)
            mm_group(1)

        self._split_multi_waits(nc)
        return nc

    def _split_multi_waits(self, nc, maxw=1):
        """walrus in this container caps sync-waits per instruction at 2;
        move excess waits onto inserted same-engine NoOps."""
        mybir = self.mybir
        for f in nc.m.functions:
            for bb in f.blocks:
                new = []
                changed = False
                for ins in list(bb.instructions):
                    si = ins.sync_info
                    waits = list(si.on_wait) if si and si.on_wait else []
                    if len(waits) > maxw:
                        changed = True
                        extra, keep = waits[:-maxw], waits[-maxw:]
                        for j in range(0, len(extra), maxw):
                            new.append(mybir.InstNoOp(
                                name=f"{ins.name}_sw{j}", engine=ins.engine,
                                sync_info=mybir.SyncInfo(
                                    on_wait=list(extra[j:j + maxw]), on_update=[]),
                                text_hint="split_wait"))
                        si.on_wait.clear()
                        for w in keep:
                            si.on_wait.append(w)
                    new.append(ins)
                if changed:
                    bb.instructions = new


def _prepare_host(labels, emb_w, w1, b1, w2):
    mx = float(np.nanmax(np.abs(labels)))
    if not np.isfinite(mx) or mx <= 0:
        mx = 1.0
    S = 1.02 * mx
    chebT, alpha = _fit_cheb(emb_w, w1, b1, w2, S)

    warp = np.zeros((128, 2), np.float32)
    for q in range(128):
        d = q // 16
        warp[q, 0] = alpha[d] / S
        warp[q, 1] = 2.0 / np.tanh(alpha[d])

    # per-core transposed-dense labels: row 16*d + c = labels[c*1024:(c+1)*1024, d]
    lab_td = []
    for c in range(_NCORES):
        lc = labels[c * _BC:(c + 1) * _BC]               # [BC, 8]
        td = lc.reshape(16, 1024, 8).transpose(2, 0, 1).reshape(128, 1024)
        lab_td.append(np.ascontiguousarray(td, dtype=np.float16))

    ones = np.ones((1, _BC), np.float16)
    const_map = {"chebT": chebT, "warp": warp, "onesr": ones}
    return S, lab_td, const_map


def _run_device(labels, emb_w, w1, b1, w2, trace=False):
    S, lab_td, const_map = _prepare_host(labels, emb_w, w1, b1, w2)
    builder = _Builder()
    nc = builder.build()

    from concourse.bass_utils import run_bass_kernel_spmd
    in_maps = [{"lab_td": lab_td[c], **const_map} for c in range(_NCORES)]
    res = run_bass_kernel_spmd(
        nc, in_maps, list(range(_NCORES)), trace=trace
    )
    out = np.empty((_B, _H), np.float32)
    for c in range(_NCORES):
        ot = res.results[c]["outT"].reshape(128, _BC)
        out[c * _BC:(c + 1) * _BC] = ot.T.astype(np.float32)
    return out, res


def kernel(labels, emb_w, w1, b1, w2, train, unconditioned):
    labels = np.asarray(labels)
    emb_w = np.asarray(emb_w, dtype=np.float32)
    w1 = np.asarray(w1, dtype=np.float32)
    b1 = np.asarray(b1, dtype=np.float32)
    w2 = np.asarray(w2, dtype=np.float32)
    if int(np.asarray(train)) or int(np.asarray(unconditioned)):
        return _np_reference(labels, emb_w, w1, b1, w2,
                             int(np.asarray(train)), int(np.asarray(unconditioned)))
    out, _ = _run_device(labels, emb_w, w1, b1, w2, trace=False)
    return out


if __name__ == "__main__":
    pass
